# revision 30
# baseline (speedup 1.0000x reference)
"""Multi-head self-attention (B=4, S=4096, D=128, H=4, no scaling, no mask)
on 8 Trainium2 NeuronCores.

Sharding: 16 (batch, head) pairs over 8 cores -> core c handles batch c//2,
heads 2*(c%2) and 2*(c%2)+1. No cross-core communication.

Per-core algorithm (flash-style, scores never touch DRAM):
  The k-projection is folded into the scores matmul:
      s_eff[k, q] = (q_q + bq) . k_k = x_k^T (Wk^T Wq x_q + Wk^T bq)
  so with y = A*(Wk^T Wq x + Wk^T bq) precomputed per head (A = 128*log2 e),
  scoresT t = A*s comes from matmul(lhsT=x_chunk[128d,128k], rhs=y[:,q512]).
  The k-bias is dropped entirely (softmax-invariant).
  exp, alternating engines per 2-chunk group:
      ACT:  pt_bf16 = Exp(t / A)                    (table exp)
      DVE:  pt_i16  = int16(t + B16) bitcast bf16   (Schraudolph: the int16
            bits ARE the bf16 of e^s up to ~4% sawtooth error)
  PV transposed in bf16: av[q=128, 33] += pt_chunk^T @ vhat_j (N=33/matmul;
  vhat carries a ones column so av[:,32] is the softmax denominator).
  PSUM accumulation windows cannot interleave within a bank, so the four
  per-u PV chains run sequentially against one av bank; each q-block's 16
  exp tiles are retained and the PV chains burst interleaved into the next
  q-block's score loop.
  normalization: ACT copies av psum->SBUF, DVE reciprocal_approx_fast on the
  four denominator columns, GpSimd per-partition-scalar multiply, DMA out.
Host gathers OUT [2, S, 32] per core into the full (B, S, D) output.
"""

import sys

for _p in ("/opt/trn_rl_repo", "/root/.axon_site/_ro/trn_rl_repo"):
    if _p not in sys.path:
        sys.path.append(_p)

import numpy as np
from contextlib import ExitStack

import concourse.bass as bass
import concourse.bacc as bacc
import concourse.mybir as mybir
import concourse.tile as tile
from concourse import bass_utils

F32 = mybir.dt.float32
F32R = mybir.dt.float32r
BF16 = mybir.dt.bfloat16
I16 = mybir.dt.int16
AF = mybir.ActivationFunctionType
ALU = mybir.AluOpType

B, D, H, HD = 4, 128, 4, 32
NCORES = 8

A_SCALE = float(np.float32(128.0 / np.log(2.0)))  # t = A*s
SCALE_INV = float(np.float32(np.log(2.0) / 128.0))  # ACT: exp(t*SCALE_INV)
B16C = float(np.float32(127.0 * 128.0 - 4.456))  # Schraudolph bf16 bias

_built = {}


def build_nc(S):
    """Build + compile the per-core program (identical across cores)."""
    NJ = S // 128  # 128-wide k-chunks
    NQB = S // 512  # 512-wide q-blocks
    NG = NJ // 2  # 2-chunk score groups per (h, q-block)
    TW = 512
    NCH = S // TW

    nc = bacc.Bacc("TRN2", target_bir_lowering=False, debug=False)

    XT = nc.dram_tensor("XT", [128, S], F32, kind="ExternalInput").ap()
    WBLOB = nc.dram_tensor("WBLOB", [128, 456], F32, kind="ExternalInput").ap()
    OUT = nc.dram_tensor("OUT", [2, S, 32], F32, kind="ExternalOutput").ap()

    with tile.TileContext(nc) as tc, ExitStack() as ctx:
        const = ctx.enter_context(tc.tile_pool(name="const", bufs=1))
        big = ctx.enter_context(tc.tile_pool(name="big", bufs=1))

        blob = const.tile([128, 456], F32R, tag="blob")
        nc.sync.dma_start(blob[:], WBLOB.bitcast(F32R))
        xts = []
        _dmaq = [nc.sync, nc.scalar]
        for ci in range(NCH):
            t = big.tile([128, TW], F32R, tag=f"xt{ci}", name=f"xt{ci}")
            _dmaq[ci % 2].dma_start(t[:], XT[:, ci * TW : (ci + 1) * TW].bitcast(F32R))
            xts.append(t)

        def xsl(col, w):
            ci = col // TW
            return xts[ci][:, col - ci * TW : col - ci * TW + w]

        mT = [blob[:, 128 * h : 128 * (h + 1)] for h in range(2)]
        wva = blob[:, 256:322]
        bvb2 = blob[:, 322:454].bitcast(F32)
        ybias = [blob[:, 454 + h : 455 + h].bitcast(F32) for h in range(2)]

        yh = [big.tile([128, S], F32R, tag=f"y{h}", name=f"y{h}") for h in range(2)]
        vhat = big.tile([128, NJ * 66], BF16, tag="vhat")

        with (
            tc.tile_pool(name="pss", bufs=3, space="PSUM") as pss,
            tc.tile_pool(name="psav", bufs=1, space="PSUM") as psav,
            tc.tile_pool(name="psy", bufs=1, space="PSUM") as psy,
            tc.tile_pool(name="work", bufs=24) as work,
            tc.tile_pool(name="nrm", bufs=3) as nrm,
        ):
            def v_pair(j):
                # chunks j, j+1 into one psum tile (two complete matmul
                # windows), one fused bias-add+bf16 copy
                pv = pss.tile([128, 1024], F32, tag="s", name=f"pv{j}")
                for t in range(2):
                    nc.tensor.matmul(
                        pv[:, 66 * t : 66 * t + 66], xsl((j + t) * 128, 128), wva,
                        start=True, stop=True,
                    )
                nc.vector.tensor_tensor(
                    out=vhat[:, j * 66 : (j + 2) * 66], in0=pv[:, 0:132], in1=bvb2,
                    op=ALU.add,
                )

            def y_chunk(h, n):
                py = psy.tile([128, 512], F32, tag="y", name=f"py{h}_{n}")
                nc.tensor.matmul(
                    py[:], mT[h], xsl(n * 512, 512), start=True, stop=True
                )
                dst = yh[h][:, n * 512 : (n + 1) * 512]
                if h == 0:
                    nc.scalar.activation(dst, py[:], AF.Identity, bias=ybias[h])
                else:
                    nc.vector.tensor_scalar(
                        out=dst, in0=py[:], scalar1=ybias[h], scalar2=None,
                        op0=ALU.add,
                    )

            def emit_norm(av, h, q0):
                avsb = nrm.tile([128, 132], F32, tag="avsb", name=f"as{h}_{q0}")
                nc.scalar.copy(avsb[:], av[:])
                rcp = nrm.tile([128, 4], F32, tag="rcp", name=f"rc{h}_{q0}")
                dens = avsb[:].rearrange("p (u c) -> p u c", c=33)[:, :, 32:33]
                nc.vector.reciprocal_approx_fast(rcp[:], dens)
                osb = nrm.tile([128, 128], F32, tag="osb", name=f"ob{h}_{q0}")
                for u in range(4):
                    nc.gpsimd.tensor_scalar(
                        out=osb[:, 32 * u : 32 * u + 32],
                        in0=avsb[:, 33 * u : 33 * u + 32],
                        scalar1=rcp[:, u : u + 1],
                        scalar2=None,
                        op0=ALU.mult,
                    )
                nc.sync.dma_start(
                    OUT[h][q0 : q0 + 512, :].rearrange("(u p) d -> p u d", p=128),
                    osb[:].rearrange("p (u d) -> p u d", d=32),
                )

            def make_u_chain(u, pts, av, h, q0, half):
                def fn():
                    for g in range(half * NG // 2, (half + 1) * NG // 2):
                        for r in range(2):
                            j = 2 * g + r
                            nc.tensor.matmul(
                                av[:, 33 * u : 33 * u + 33],
                                pts[g][:, 512 * r + 128 * u : 512 * r + 128 * u + 128].bitcast(BF16),
                                vhat[:, 66 * j + 33 * h : 66 * j + 33 * h + 33],
                                start=(g == 0 and r == 0),
                                stop=(g == NG - 1 and r == 1),
                            )
                    if u == 3 and half == 1:
                        emit_norm(av, h, q0)

                return fn

            # exp-table preload before the first pipelined Exp
            scratch = const.tile([1, 1], F32, tag="scr")
            nc.scalar.activation(scratch[:], blob[0:1, 454:455].bitcast(F32), AF.Exp,
                                 scale=SCALE_INV)

            pending_pv = []
            y_chunk(0, 0)
            v_pair(0)
            v_pair(2)
            for h in range(2):
                for i0 in range(NQB):
                    q0 = 512 * i0
                    av = psav.tile([128, 132], F32, tag="av", name=f"av{h}_{q0}")
                    pts = []
                    for g in range(NG):
                        if h == 0 and i0 == 0 and g < 12:
                            # spread the v projection through the first q-blocks
                            v_pair(4 + 2 * g)
                        elif h == 0 and i0 == 1 and g < 2:
                            v_pair(28 + 2 * g)
                        ps = pss.tile([128, 1024], F32, tag="s", name=f"s{h}_{q0}_{g}")
                        for r in range(2):
                            nc.tensor.matmul(
                                ps[:, 512 * r : 512 * (r + 1)],
                                xsl((2 * g + r) * 128, 128),
                                yh[h][:, q0 : q0 + 512],
                                start=True,
                                stop=True,
                            )
                        # alternate exp engines; ACT gets an extra group on
                        # even q-blocks (8.5/7.5 average split)
                        if i0 % 2 == 0 or i0 == 7:
                            eng = 0 if (g % 2 == 0) != (g >= 7) else 1
                        else:
                            eng = g % 2
                        pt = work.tile([128, 1024], I16, tag="pt", name=f"pt{h}_{q0}_{g}")
                        if eng == 0:
                            nc.scalar.activation(
                                pt[:].bitcast(BF16), ps[:], AF.Exp, scale=SCALE_INV
                            )
                        else:
                            nc.vector.tensor_scalar(
                                out=pt[:], in0=ps[:], scalar1=B16C, scalar2=None,
                                op0=ALU.add,
                            )
                        pts.append(pt)
                        if pending_pv and g % 2 == 1:
                            pending_pv.pop(0)()
                        # stage upcoming y projections in the even-qb
                        # double-ACT bubble (g6/g7) where DVE has slack
                        if i0 % 2 == 0 and g in (6, 7):
                            n = i0 + 1 + (g - 6)
                            if n < NQB:
                                y_chunk(h, n)
                            elif h == 0:
                                y_chunk(1, 0)
                    pending_pv = [make_u_chain(u, pts, av, h, q0, hf)
                                  for u in range(4) for hf in (0, 1)]
            for fn in pending_pv:
                fn()

    nc.compile()
    return nc


def _host_prep(x, Wq, bq, Wk, bk, Wv, bv, S):
    """Per-core input maps."""
    in_maps = []
    for c in range(NCORES):
        b, hp = c // 2, c % 2
        h0 = 2 * hp
        xt = np.ascontiguousarray(x[b].T).astype(np.float32)  # [128, S]
        blob = np.zeros((128, 456), np.float32)
        for i in range(2):
            hh = h0 + i
            wq_h = Wq[hh * 32 : (hh + 1) * 32, :].astype(np.float64)
            wk_h = Wk[hh * 32 : (hh + 1) * 32, :].astype(np.float64)
            bq_h = bq[hh * 32 : (hh + 1) * 32].astype(np.float64)
            # y = A*(Wk^T Wq x + Wk^T bq); lhsT for y-proj is (Wk^T Wq)^T = Wq^T Wk
            blob[:, 128 * i : 128 * (i + 1)] = (A_SCALE * (wq_h.T @ wk_h)).astype(np.float32)
            blob[:, 454 + i] = (A_SCALE * (wk_h.T @ bq_h)).astype(np.float32)
            blob[:, 256 + 33 * i : 256 + 33 * i + 32] = Wv[hh * 32 : (hh + 1) * 32, :].T
            for rep in range(2):
                c0 = 322 + 66 * rep + 33 * i
                blob[:, c0 : c0 + 32] = bv[hh * 32 : (hh + 1) * 32][None, :]
                blob[:, c0 + 32] = 1.0
        in_maps.append({"XT": xt, "WBLOB": blob})
    return in_maps


def _unshard(results, S):
    out = np.empty((B, S, D), np.float32)
    for c in range(NCORES):
        b, hp = c // 2, c % 2
        oc = results[c]["OUT"]  # [2, S, 32]
        for hl in range(2):
            hh = 2 * hp + hl
            out[b, :, hh * 32 : (hh + 1) * 32] = oc[hl]
    return out


def _run_once(args):
    x, Wq, bq, Wk, bk, Wv, bv = args
    S = x.shape[1]
    if S not in _built:
        _built[S] = build_nc(S)
    nc = _built[S]
    in_maps = _host_prep(x, Wq, bq, Wk, bk, Wv, bv, S)
    res = bass_utils.run_bass_kernel_spmd(nc, in_maps, core_ids=list(range(NCORES)))
    return _unshard(res.results, S)


def _subproc_entry(args):
    return _run_once(args)


def kernel(x, Wq, bq, Wk, bk, Wv, bv):
    args = tuple(
        np.asarray(a, dtype=np.float32) for a in (x, Wq, bq, Wk, bk, Wv, bv)
    )
    # The axon/NRT stack occasionally fails a first dispatch with
    # NRT_EXEC_UNIT_UNRECOVERABLE (device auto-recovers). Retry in-process,
    # then in a fresh spawned process (compile caches make that cheap).
    try:
        return _run_once(args)
    except Exception:
        try:
            return _run_once(args)
        except Exception:
            import multiprocessing as mp

            ctx = mp.get_context("spawn")
            with ctx.Pool(1) as pool:
                return pool.apply(_subproc_entry, (args,))


# revision 31
# speedup vs baseline: 1.0060x; 1.0060x over previous
"""Multi-head self-attention (B=4, S=4096, D=128, H=4, no scaling, no mask)
on 8 Trainium2 NeuronCores.

Sharding: 16 (batch, head) pairs over 8 cores -> core c handles batch c//2,
heads 2*(c%2) and 2*(c%2)+1. No cross-core communication.

Per-core algorithm (flash-style, scores never touch DRAM):
  The k-projection is folded into the scores matmul:
      s_eff[k, q] = (q_q + bq) . k_k = x_k^T (Wk^T Wq x_q + Wk^T bq)
  so with y = A*(Wk^T Wq x + Wk^T bq) precomputed per head (A = 128*log2 e),
  scoresT t = A*s comes from matmul(lhsT=x_chunk[128d,128k], rhs=y[:,q512]).
  The k-bias is dropped entirely (softmax-invariant).
  exp, alternating engines per 2-chunk group:
      ACT:  pt_bf16 = Exp(t / A)                    (table exp)
      DVE:  pt_i16  = int16(t + B16) bitcast bf16   (Schraudolph: the int16
            bits ARE the bf16 of e^s up to ~4% sawtooth error)
  PV transposed in bf16: av[q=128, 33] += pt_chunk^T @ vhat_j (N=33/matmul;
  vhat carries a ones column so av[:,32] is the softmax denominator).
  PSUM accumulation windows cannot interleave within a bank, so the four
  per-u PV chains run sequentially against one av bank; each q-block's 16
  exp tiles are retained and the PV chains burst interleaved into the next
  q-block's score loop.
  normalization: ACT copies av psum->SBUF, DVE reciprocal_approx_fast on the
  four denominator columns, GpSimd per-partition-scalar multiply, DMA out.
Host gathers OUT [2, S, 32] per core into the full (B, S, D) output.
"""

import sys

for _p in ("/opt/trn_rl_repo", "/root/.axon_site/_ro/trn_rl_repo"):
    if _p not in sys.path:
        sys.path.append(_p)

import numpy as np
from contextlib import ExitStack

import concourse.bass as bass
import concourse.bacc as bacc
import concourse.mybir as mybir
import concourse.tile as tile
from concourse import bass_utils

F32 = mybir.dt.float32
F32R = mybir.dt.float32r
BF16 = mybir.dt.bfloat16
I16 = mybir.dt.int16
AF = mybir.ActivationFunctionType
ALU = mybir.AluOpType

B, D, H, HD = 4, 128, 4, 32
NCORES = 8

A_SCALE = float(np.float32(128.0 / np.log(2.0)))  # t = A*s
SCALE_INV = float(np.float32(np.log(2.0) / 128.0))  # ACT: exp(t*SCALE_INV)
B16C = float(np.float32(127.0 * 128.0 - 4.456))  # Schraudolph bf16 bias

_built = {}


def build_nc(S):
    """Build + compile the per-core program (identical across cores)."""
    NJ = S // 128  # 128-wide k-chunks
    NQB = S // 512  # 512-wide q-blocks
    NG = NJ // 2  # 2-chunk score groups per (h, q-block)
    TW = 512
    NCH = S // TW

    nc = bacc.Bacc("TRN2", target_bir_lowering=False, debug=False)

    XT = nc.dram_tensor("XT", [128, S], F32, kind="ExternalInput").ap()
    WBLOB = nc.dram_tensor("WBLOB", [128, 456], F32, kind="ExternalInput").ap()
    OUT = nc.dram_tensor("OUT", [2, S, 32], F32, kind="ExternalOutput").ap()

    with tile.TileContext(nc) as tc, ExitStack() as ctx:
        const = ctx.enter_context(tc.tile_pool(name="const", bufs=1))
        big = ctx.enter_context(tc.tile_pool(name="big", bufs=1))

        blob = const.tile([128, 456], F32R, tag="blob")
        nc.sync.dma_start(blob[:], WBLOB.bitcast(F32R))
        xts = []
        _dmaq = [nc.sync, nc.scalar]
        for ci in range(NCH):
            t = big.tile([128, TW], F32R, tag=f"xt{ci}", name=f"xt{ci}")
            _dmaq[ci % 2].dma_start(t[:], XT[:, ci * TW : (ci + 1) * TW].bitcast(F32R))
            xts.append(t)

        def xsl(col, w):
            ci = col // TW
            return xts[ci][:, col - ci * TW : col - ci * TW + w]

        mT = [blob[:, 128 * h : 128 * (h + 1)] for h in range(2)]
        wva = blob[:, 256:322]
        bvb2 = blob[:, 322:454].bitcast(F32)
        ybias = [blob[:, 454 + h : 455 + h].bitcast(F32) for h in range(2)]

        yh = [big.tile([128, S], F32R, tag=f"y{h}", name=f"y{h}") for h in range(2)]
        vhat = big.tile([128, NJ * 66], BF16, tag="vhat")

        with (
            tc.tile_pool(name="pss", bufs=3, space="PSUM") as pss,
            tc.tile_pool(name="psav", bufs=1, space="PSUM") as psav,
            tc.tile_pool(name="psy", bufs=1, space="PSUM") as psy,
            tc.tile_pool(name="work", bufs=24) as work,
            tc.tile_pool(name="nrm", bufs=3) as nrm,
        ):
            def v_pair(j):
                # chunks j, j+1 into one psum tile (two complete matmul
                # windows), one fused bias-add+bf16 copy
                pv = pss.tile([128, 1024], F32, tag="s", name=f"pv{j}")
                for t in range(2):
                    nc.tensor.matmul(
                        pv[:, 66 * t : 66 * t + 66], xsl((j + t) * 128, 128), wva,
                        start=True, stop=True,
                    )
                nc.vector.tensor_tensor(
                    out=vhat[:, j * 66 : (j + 2) * 66], in0=pv[:, 0:132], in1=bvb2,
                    op=ALU.add,
                )

            def y_chunk(h, n):
                py = psy.tile([128, 512], F32, tag="y", name=f"py{h}_{n}")
                nc.tensor.matmul(
                    py[:], mT[h], xsl(n * 512, 512), start=True, stop=True
                )
                dst = yh[h][:, n * 512 : (n + 1) * 512]
                if h == 0:
                    nc.scalar.activation(dst, py[:], AF.Identity, bias=ybias[h])
                else:
                    nc.vector.tensor_scalar(
                        out=dst, in0=py[:], scalar1=ybias[h], scalar2=None,
                        op0=ALU.add,
                    )

            def emit_norm(av, h, q0):
                avsb = nrm.tile([128, 132], F32, tag="avsb", name=f"as{h}_{q0}")
                nc.scalar.copy(avsb[:], av[:])
                rcp = nrm.tile([128, 4], F32, tag="rcp", name=f"rc{h}_{q0}")
                dens = avsb[:].rearrange("p (u c) -> p u c", c=33)[:, :, 32:33]
                nc.vector.reciprocal_approx_fast(rcp[:], dens)
                osb = nrm.tile([128, 128], F32, tag="osb", name=f"ob{h}_{q0}")
                for u in range(4):
                    nc.gpsimd.tensor_scalar(
                        out=osb[:, 32 * u : 32 * u + 32],
                        in0=avsb[:, 33 * u : 33 * u + 32],
                        scalar1=rcp[:, u : u + 1],
                        scalar2=None,
                        op0=ALU.mult,
                    )
                nc.sync.dma_start(
                    OUT[h][q0 : q0 + 512, :].rearrange("(u p) d -> p u d", p=128),
                    osb[:].rearrange("p (u d) -> p u d", d=32),
                )

            def make_u_chain(u, pts, av, h, q0, half):
                def fn():
                    for g in range(half * NG // 2, (half + 1) * NG // 2):
                        for r in range(2):
                            j = 2 * g + r
                            nc.tensor.matmul(
                                av[:, 33 * u : 33 * u + 33],
                                pts[g][:, 512 * r + 128 * u : 512 * r + 128 * u + 128].bitcast(BF16),
                                vhat[:, 66 * j + 33 * h : 66 * j + 33 * h + 33],
                                start=(g == 0 and r == 0),
                                stop=(g == NG - 1 and r == 1),
                            )
                    if u == 3 and half == 1:
                        emit_norm(av, h, q0)

                return fn

            # exp-table preload before the first pipelined Exp
            scratch = const.tile([1, 1], F32, tag="scr")
            nc.scalar.activation(scratch[:], blob[0:1, 454:455].bitcast(F32), AF.Exp,
                                 scale=SCALE_INV)

            pending_pv = []
            y_chunk(0, 0)
            v_pair(0)
            v_pair(2)
            for h in range(2):
                for i0 in range(NQB):
                    q0 = 512 * i0
                    av = psav.tile([128, 132], F32, tag="av", name=f"av{h}_{q0}")
                    pts = []
                    for g in range(NG):
                        if h == 0 and i0 == 0 and g < 12:
                            # spread the v projection through the first q-blocks
                            v_pair(4 + 2 * g)
                        elif h == 0 and i0 == 1 and g < 2:
                            v_pair(28 + 2 * g)
                        ps = pss.tile([128, 1024], F32, tag="s", name=f"s{h}_{q0}_{g}")
                        for r in range(2):
                            nc.tensor.matmul(
                                ps[:, 512 * r : 512 * (r + 1)],
                                xsl((2 * g + r) * 128, 128),
                                yh[h][:, q0 : q0 + 512],
                                start=True,
                                stop=True,
                            )
                        # alternate exp engines; ACT gets an extra group on
                        # even q-blocks (8.5/7.5 average split)
                        if i0 % 2 == 0:
                            eng = 0 if (g % 2 == 0) != (g >= 7) else 1
                        else:
                            eng = g % 2
                        pt = work.tile([128, 1024], I16, tag="pt", name=f"pt{h}_{q0}_{g}")
                        if eng == 0:
                            nc.scalar.activation(
                                pt[:].bitcast(BF16), ps[:], AF.Exp, scale=SCALE_INV
                            )
                        else:
                            nc.vector.tensor_scalar(
                                out=pt[:], in0=ps[:], scalar1=B16C, scalar2=None,
                                op0=ALU.add,
                            )
                        pts.append(pt)
                        if pending_pv and g % 2 == 1:
                            pending_pv.pop(0)()
                        # stage upcoming y projections in the even-qb
                        # double-ACT bubble (g6/g7) where DVE has slack
                        if i0 % 2 == 0 and g in (6, 7):
                            n = i0 + 1 + (g - 6)
                            if n < NQB:
                                y_chunk(h, n)
                            elif h == 0:
                                y_chunk(1, 0)
                    pending_pv = [make_u_chain(u, pts, av, h, q0, hf)
                                  for u in range(4) for hf in (0, 1)]
            for fn in pending_pv:
                fn()

    nc.compile()
    return nc


def _host_prep(x, Wq, bq, Wk, bk, Wv, bv, S):
    """Per-core input maps."""
    in_maps = []
    for c in range(NCORES):
        b, hp = c // 2, c % 2
        h0 = 2 * hp
        xt = np.ascontiguousarray(x[b].T).astype(np.float32)  # [128, S]
        blob = np.zeros((128, 456), np.float32)
        for i in range(2):
            hh = h0 + i
            wq_h = Wq[hh * 32 : (hh + 1) * 32, :].astype(np.float64)
            wk_h = Wk[hh * 32 : (hh + 1) * 32, :].astype(np.float64)
            bq_h = bq[hh * 32 : (hh + 1) * 32].astype(np.float64)
            # y = A*(Wk^T Wq x + Wk^T bq); lhsT for y-proj is (Wk^T Wq)^T = Wq^T Wk
            blob[:, 128 * i : 128 * (i + 1)] = (A_SCALE * (wq_h.T @ wk_h)).astype(np.float32)
            blob[:, 454 + i] = (A_SCALE * (wk_h.T @ bq_h)).astype(np.float32)
            blob[:, 256 + 33 * i : 256 + 33 * i + 32] = Wv[hh * 32 : (hh + 1) * 32, :].T
            for rep in range(2):
                c0 = 322 + 66 * rep + 33 * i
                blob[:, c0 : c0 + 32] = bv[hh * 32 : (hh + 1) * 32][None, :]
                blob[:, c0 + 32] = 1.0
        in_maps.append({"XT": xt, "WBLOB": blob})
    return in_maps


def _unshard(results, S):
    out = np.empty((B, S, D), np.float32)
    for c in range(NCORES):
        b, hp = c // 2, c % 2
        oc = results[c]["OUT"]  # [2, S, 32]
        for hl in range(2):
            hh = 2 * hp + hl
            out[b, :, hh * 32 : (hh + 1) * 32] = oc[hl]
    return out


def _run_once(args):
    x, Wq, bq, Wk, bk, Wv, bv = args
    S = x.shape[1]
    if S not in _built:
        _built[S] = build_nc(S)
    nc = _built[S]
    in_maps = _host_prep(x, Wq, bq, Wk, bk, Wv, bv, S)
    res = bass_utils.run_bass_kernel_spmd(nc, in_maps, core_ids=list(range(NCORES)))
    return _unshard(res.results, S)


def _subproc_entry(args):
    return _run_once(args)


def kernel(x, Wq, bq, Wk, bk, Wv, bv):
    args = tuple(
        np.asarray(a, dtype=np.float32) for a in (x, Wq, bq, Wk, bk, Wv, bv)
    )
    # The axon/NRT stack occasionally fails a first dispatch with
    # NRT_EXEC_UNIT_UNRECOVERABLE (device auto-recovers). Retry in-process,
    # then in a fresh spawned process (compile caches make that cheap).
    try:
        return _run_once(args)
    except Exception:
        try:
            return _run_once(args)
        except Exception:
            import multiprocessing as mp

            ctx = mp.get_context("spawn")
            with ctx.Pool(1) as pool:
                return pool.apply(_subproc_entry, (args,))
